# revision 1
# baseline (speedup 1.0000x reference)
"""DAV_Block cost-volume kernel for Trainium2 (8 NeuronCores, SPMD).

Computes sim[b,d,h,w] = cosine similarity between 3x3xC patches of q and
warped_feat[..., d]:
    qq  = box3(sum_c q^2);  kk = box3(sum_c wf_d^2);  num = box3(sum_c q*wf_d)
    sim = num / (max(sqrt(qq),eps) * max(sqrt(kk),eps))

Sharding: 8 cores = b(2) x h-quarter(4).  Each core gets a 48-row h-slice
(+1 halo row each side, zeros at global edges) with all C, W, D.

Per-core dataflow (fp32 in, bf16 through the PE):
  partitions = (h-pair, c) = 128
  ACT: sq = wf^2          -> bf16
  DVE: pr = wf * q_bcast  -> bf16
  PE : banded ones lhsT [128, 48] (bf16 -- fp32r weights would lower to
       IndirectLoad weight DMAs whose 16-bit semaphore overflows at ~1k
       matmuls) performs channel-sum AND the 3-tap h-box in one
       accumulation chain; PSUM accumulates fp32.

Schedule (driven by a TimelineSim study of the previous version):
  * w is processed in 6 groups of [32,64,64,64,64,32] cols; per (group,t)
    one wf DMA [128p, cols x 32d] on the SP HWDGE ring (engines are
    in-order, so ACT/DVE never issue or wait on wf DMAs).
  * group 0 is 32 cols so its PSUM accs (4 banks) coexist with qq_acc
    (1 bank): phase A (qq) interleaves into group 0's t-loop with no
    PE stall.  Phase A's elementwise work runs on the otherwise-idle
    gpsimd engine so ACT/DVE start main-loop work immediately.
  * phase C (w-box + normalize + output DMA) is chunked; chunk k is
    emitted right after the group that supplies its last halo col and
    drained one op per t-iteration into the next group's loop, so no
    engine sees a burst.  Box/prod ops go to gpsimd (idle), recip/mul
    to DVE, sqrt to ACT.  Only a 33-col chunk remains after the last
    group -> ~10 us tail instead of ~50 us.
"""
import numpy as np
from contextlib import ExitStack

import concourse.bass as bass
from concourse import bacc
import concourse.tile as tile
from concourse import mybir
from concourse.bass_utils import run_bass_kernel_spmd

# Problem shape (hardcoded per contest contract)
B, C, H, W, D = 2, 64, 192, 320, 32
NCORES = 8
HQ = 4                 # h-quarters per batch
HOUT = H // HQ         # 48 out rows per core
HIN = HOUT + 2         # 50 input rows (1 halo each side)
NT = HIN // 2          # 25 h-pairs
J0 = HOUT              # center col of the banded weight pattern
GW = 2 * HOUT          # G width: cols [0, 96)
WHALF = W // 2         # 160
FSTG = (WHALF + 2) * D  # stage free size incl. 1 halo col each side: 162*32
MAXW = 64              # max group width (cols)
MAXCHUNK = 95          # max phase-C chunk width (cols)

# (start, width): first/last groups are 32 cols so their 4-bank PSUM accs
# leave room for qq_acc (group 0) / reuse freed banks (group 5)
GROUPS = [(0, 32), (32, 64), (96, 64), (160, 64), (224, 64), (288, 32)]
# phase-C chunks: (out-col start, out-col end, ready-after-group).  Chunk
# [a,b) needs stage cols a-1..b, i.e. the evacuation of the group holding
# col b (col 320 is the memset right edge halo).
CHUNKS = [(0, 95, 1), (95, 159, 2), (159, 223, 3), (223, 287, 4), (287, 320, 5)]

_NC_CACHE = None


def _build_nc():
    nc = bacc.Bacc(None, target_bir_lowering=False)
    wf_d = nc.declare_dram_parameter("wf", [C, HIN, W, D], mybir.dt.float32, isOutput=False)
    q_d = nc.declare_dram_parameter("q", [C, HIN, W], mybir.dt.float32, isOutput=False)
    g_d = nc.declare_dram_parameter("g", [128, GW], mybir.dt.float32, isOutput=False)
    o_d = nc.declare_dram_parameter("o", [HOUT, W, D], mybir.dt.float32, isOutput=True)

    f32 = mybir.dt.float32
    bf16 = mybir.dt.bfloat16
    SQ = mybir.ActivationFunctionType.Square

    with ExitStack() as ctx:
        tc = ctx.enter_context(tile.TileContext(nc))
        cpool = ctx.enter_context(tc.tile_pool(name="const", bufs=1))
        spool = ctx.enter_context(tc.tile_pool(name="stage", bufs=1))
        pool = ctx.enter_context(tc.tile_pool(name="work", bufs=6))
        epool = ctx.enter_context(tc.tile_pool(name="elem", bufs=4))

        # ---- constants (scalar HWDGE ring; sync ring is reserved for wf) ----
        g = cpool.tile([128, GW], f32)
        nc.scalar.dma_start(g[:], g_d[:])
        gb = cpool.tile([128, GW], bf16)
        nc.scalar.copy(gb[:], g[:])

        # q resident: [128p=(hpair,c), t, w] -- loaded per-t so phase A can
        # start as soon as the first rows land
        q_all = cpool.tile([128, NT, W], f32)
        q_src = q_d[:].rearrange("c (t hp) w -> hp c t w", hp=2)
        for t in range(NT):
            nc.scalar.dma_start(q_all[:, t, :], q_src[:, :, t, :])

        # ---- stage + scratch tensors ----
        num_stg = spool.tile([112, FSTG], f32)
        kk_stg = spool.tile([112, FSTG], f32)
        qq_stg = spool.tile([112, WHALF + 2], f32)
        qq_box = spool.tile([112, WHALF], f32)
        # phase-C chunk scratch (persistent; chunks use [quadrant, :wd*D])
        bnum_s = spool.tile([112, MAXCHUNK * D], f32)
        bkk_s = spool.tile([112, MAXCHUNK * D], f32)
        p1_s = spool.tile([112, MAXCHUNK * D], f32)
        p2_s = spool.tile([112, MAXCHUNK * D], f32)

        # edge halos (independent of everything; gpsimd is idle at t=0)
        nc.gpsimd.memset(num_stg[0:HOUT, 0:D], 0.0)
        nc.gpsimd.memset(kk_stg[0:HOUT, 0:D], 0.0)
        nc.gpsimd.memset(num_stg[64 : 64 + HOUT, (WHALF + 1) * D : FSTG], 0.0)
        nc.gpsimd.memset(kk_stg[64 : 64 + HOUT, (WHALF + 1) * D : FSTG], 0.0)
        nc.gpsimd.memset(qq_stg[0:HOUT, 0:1], 0.0)
        nc.gpsimd.memset(qq_stg[64 : 64 + HOUT, WHALF + 1 : WHALF + 2], 0.0)
        # recip/sqrt below run on partitions [0:112] (base 0) because the
        # custom DVE ISA op reciprocal_approx_fast silently no-ops when its
        # AP partition base is 64 (HW-probed; CoreSim models it fine).  Fill
        # the dead 48:64 band so those lanes read initialized data.
        nc.gpsimd.memset(p1_s[0:112, :], 1.0)
        nc.gpsimd.memset(p2_s[0:112, :], 1.0)

        # queued (weight, op) phase-C closures.  Engines are in-order, so a
        # queued op that still waits on its predecessor blocks everything
        # behind it on that engine: ops must be spaced out so each link of
        # the serial chain finishes before the next engine reaches it.
        # 1 credit accrues per t-iteration; heavy (>=33 col) ops cost 2.
        pending = []
        drain_credit = [0]

        def drain(credits):
            drain_credit[0] += credits
            while pending and drain_credit[0] >= pending[0][0]:
                w, op = pending.pop(0)
                drain_credit[0] -= w
                op()

        def emit_chunk(a, b, tail=False, soff=0):
            """Box-w + normalize + output DMA for out cols [a, b).

            Queues (weight, closure) pairs on `pending` (returns the op
            list if tail).  [a,b) must lie in one w-half.  Box adds and
            prod run on gpsimd, where in-order blocking is harmless (no
            main-loop work queues behind them).  The normalize (sqrt ->
            recip -> mul) is emitted in 16-col strips so the ACT/DVE
            links each wait at most ~0.5us -- a big queued op that waits
            would stall every main-loop op behind it on that engine.
            """
            qd = a // WHALF
            wo = a - WHALF * qd
            wd = b - a
            ps = slice(64 * qd, 64 * qd + HOUT)
            n = wd * D
            ss = slice(soff, soff + n)
            ops = []
            box_eng = nc.vector
            bnum_eng = nc.gpsimd

            bw = 2 if wd >= 33 else (0 if wd <= 4 else 1)

            def box2(dst, src, eng):
                out = []
                out.append((bw, lambda: eng.tensor_add(
                    dst[ps, ss],
                    src[ps, wo * D : wo * D + n],
                    src[ps, (wo + 2) * D : (wo + 2) * D + n])))
                out.append((bw, lambda: eng.tensor_add(
                    dst[ps, ss], dst[ps, ss],
                    src[ps, (wo + 1) * D : (wo + 1) * D + n])))
                return out

            bk = box2(bkk_s, kk_stg, box_eng)
            bn = box2(bnum_s, num_stg, bnum_eng)
            ops += [bk[0], bn[0], bk[1], bn[1]]
            sws = 0 if wd <= 4 else 1
            for s0 in range(0, wd, 32):
                sw = min(32, wd - s0)
                m = sw * D
                st = slice(soff + s0 * D, soff + s0 * D + m)
                ops.append((sws, lambda st=st, s0=s0, sw=sw, m=m:
                    nc.gpsimd.tensor_mul(
                        p1_s[ps, st].rearrange("p (w d) -> p w d", d=D),
                        bkk_s[ps, st].rearrange("p (w d) -> p w d", d=D),
                        qq_box[ps, wo + s0 : wo + s0 + sw]
                        .unsqueeze(-1).broadcast_to([HOUT, sw, D]))))
                ops.append((sws, lambda st=st: nc.scalar.activation(
                    p2_s[0:112, st], p1_s[0:112, st],
                    mybir.ActivationFunctionType.Sqrt)))
                ops.append((sws, lambda st=st: nc.vector.reciprocal_approx_fast(
                    p1_s[0:112, st], p2_s[0:112, st])))
                ops.append((sws, lambda st=st: nc.vector.tensor_mul(
                    p2_s[ps, st], bnum_s[ps, st], p1_s[ps, st])))
            ops.append((1, lambda: nc.gpsimd.dma_start(
                o_d[:, a:b, :],
                p2_s[ps, ss].rearrange("p (w d) -> p w d", d=D))))
            if tail:
                return [op for _, op in ops]
            pending.extend(ops)
            return None

        def run_group(gi, qq_work):
            wstart, wd = GROUPS[gi]
            nfree = wd * D
            nch = nfree // 512
            acc_num = mpsum.tile([128, nfree], f32, tag=f"acc_num{wd}")
            acc_kk = mpsum.tile([128, nfree], f32, tag=f"acc_kk{wd}")
            for t in range(NT):
                wf_t = pool.tile([128, MAXW * D], f32, tag="wf")
                src = (
                    wf_d[:]
                    .rearrange("c (t hp) w d -> t hp c w d", hp=2)[t]
                    [:, :, wstart : wstart + wd, :]
                )
                nc.sync.dma_start(
                    wf_t[:, 0:nfree].rearrange("p (w d) -> p w d", d=D), src
                )

                sq_t = epool.tile([128, MAXW * D], bf16, tag="sq")
                nc.scalar.activation(sq_t[:, 0:nfree], wf_t[:, 0:nfree], SQ)

                pr_t = epool.tile([128, MAXW * D], bf16, tag="pr")
                q_b = (
                    q_all[:, t, wstart : wstart + wd]
                    .unsqueeze(-1)
                    .broadcast_to([128, wd, D])
                )
                nc.vector.tensor_mul(
                    pr_t[:, 0:nfree].rearrange("p (w d) -> p w d", d=D),
                    wf_t[:, 0:nfree].rearrange("p (w d) -> p w d", d=D),
                    q_b,
                )

                lhsT = gb[:, J0 - 2 * t : J0 - 2 * t + HOUT]
                first, last = (t == 0), (t == NT - 1)
                for ch in range(nch):
                    sl = slice(512 * ch, 512 * (ch + 1))
                    nc.tensor.matmul(acc_num[0:HOUT, sl], lhsT, pr_t[:, sl],
                                     start=first, stop=last)
                    nc.tensor.matmul(acc_kk[0:HOUT, sl], lhsT, sq_t[:, sl],
                                     start=first, stop=last)
                if qq_work:
                    # phase A rides along group 0: q DMAs run well ahead of
                    # wf DMAs, so these never block the in-order PE
                    sqq = epool.tile([128, W], bf16, tag="sqq")
                    nc.gpsimd.tensor_mul(sqq[:], q_all[:, t, :], q_all[:, t, :])
                    nc.tensor.matmul(qq_acc[0:HOUT, :], lhsT, sqq[:],
                                     start=first, stop=last)
                drain(1)

            # evacuate: psum [48, nfree] -> stage quadrant (both on ACT --
            # DVE is the scarcer engine)
            qd = wstart // WHALF
            wo = wstart - WHALF * qd
            pbase = 64 * qd
            foff = (1 + wo) * D
            nc.scalar.copy(num_stg[pbase : pbase + HOUT, foff : foff + nfree],
                           acc_num[0:HOUT, :])
            nc.scalar.copy(kk_stg[pbase : pbase + HOUT, foff : foff + nfree],
                           acc_kk[0:HOUT, :])

        # ---- main loop ----
        with tc.tile_pool(name="qq_psum", bufs=1, space="PSUM") as qpsum, \
             tc.tile_pool(name="mm32_psum", bufs=1, space="PSUM") as mpsum:
            qq_acc = qpsum.tile([128, W], f32)
            run_group(0, qq_work=True)
            # qq: evacuate (ACT reaches this ~40 us in; the qq matmul chain
            # finished at ~12 us, so no stall), halos + box on gpsimd
            nc.scalar.copy(qq_stg[0:HOUT, 1 : WHALF + 1], qq_acc[0:HOUT, 0:WHALF])
            nc.scalar.copy(qq_stg[64 : 64 + HOUT, 1 : WHALF + 1],
                           qq_acc[0:HOUT, WHALF:W])
            nc.gpsimd.tensor_copy(qq_stg[0:HOUT, WHALF + 1 : WHALF + 2],
                                  qq_stg[64 : 64 + HOUT, 1:2])
            nc.gpsimd.tensor_copy(qq_stg[64 : 64 + HOUT, 0:1],
                                  qq_stg[0:HOUT, WHALF : WHALF + 1])
            for qs in (slice(0, HOUT), slice(64, 64 + HOUT)):
                nc.gpsimd.tensor_add(qq_box[qs, :], qq_stg[qs, 0:WHALF],
                                     qq_stg[qs, 2 : WHALF + 2])
                nc.gpsimd.tensor_add(qq_box[qs, :], qq_box[qs, :],
                                     qq_stg[qs, 1 : WHALF + 1])

        with tc.tile_pool(name="mm64_psum", bufs=1, space="PSUM") as mpsum:
            for gi in (1, 2, 3, 4):
                run_group(gi, qq_work=False)
                if gi == 3:
                    # interface halos between the two w-halves (cols 159/160
                    # both evacuated now; needed by the chunks ending at 223)
                    nc.scalar.copy(num_stg[0:HOUT, (WHALF + 1) * D : FSTG],
                                   num_stg[64 : 64 + HOUT, D : 2 * D])
                    nc.scalar.copy(kk_stg[0:HOUT, (WHALF + 1) * D : FSTG],
                                   kk_stg[64 : 64 + HOUT, D : 2 * D])
                    nc.scalar.copy(num_stg[64 : 64 + HOUT, 0:D],
                                   num_stg[0:HOUT, WHALF * D : (WHALF + 1) * D])
                    nc.scalar.copy(kk_stg[64 : 64 + HOUT, 0:D],
                                   kk_stg[0:HOUT, WHALF * D : (WHALF + 1) * D])
                for a, bnd, rdy in CHUNKS:
                    if rdy == gi:
                        if a < WHALF < bnd:
                            emit_chunk(a, WHALF)
                            emit_chunk(WHALF, bnd)
                        else:
                            emit_chunk(a, bnd)

        with tc.tile_pool(name="mm32b_psum", bufs=1, space="PSUM") as mpsum:
            run_group(5, qq_work=False)

        while pending:
            pending.pop(0)[1]()
        # tail: two parallel chains on complementary engine flavors
        ops_a = emit_chunk(287, 303, tail=True)
        ops_b = emit_chunk(303, 320, tail=True, soff=16 * D)
        for op_a, op_b in zip(ops_a, ops_b):
            op_a()
            op_b()

    nc.compile()
    return nc


def _g_pattern() -> np.ndarray:
    """g[p=(hp*64+c), j] = 1 iff j - J0 in {hp-2, hp-1, hp}."""
    g = np.zeros((128, GW), dtype=np.float32)
    for hp in range(2):
        for dj in (hp - 2, hp - 1, hp):
            j = J0 + dj
            if 0 <= j < GW:
                g[hp * 64 : (hp + 1) * 64, j] = 1.0
    return g


def get_nc():
    global _NC_CACHE
    if _NC_CACHE is None:
        _NC_CACHE = _build_nc()
    return _NC_CACHE


def make_in_maps(q: np.ndarray, warped_feat: np.ndarray):
    """Marshal full inputs into 8 per-core input maps."""
    q = np.asarray(q, dtype=np.float32)
    wf = np.asarray(warped_feat, dtype=np.float32)
    g = _g_pattern()
    in_maps = []
    for core in range(NCORES):
        b, j = divmod(core, HQ)
        h0 = j * HOUT - 1          # inclusive, may be -1
        h1 = j * HOUT + HOUT + 1   # exclusive, may be H+1
        lo_pad = 1 if h0 < 0 else 0
        hi_pad = 1 if h1 > H else 0
        hs = slice(h0 + lo_pad, h1 - hi_pad)
        q_c = np.zeros((C, HIN, W), dtype=np.float32)
        q_c[:, lo_pad : HIN - hi_pad, :] = q[b][:, hs, :]
        wf_c = np.zeros((C, HIN, W, D), dtype=np.float32)
        wf_c[:, lo_pad : HIN - hi_pad, :, :] = wf[b][:, hs, :, :]
        in_maps.append({"wf": wf_c, "q": q_c, "g": g})
    return in_maps


def assemble(results) -> np.ndarray:
    out = np.empty((B, D, H, W), dtype=np.float32)
    for core in range(NCORES):
        b, j = divmod(core, HQ)
        o = results[core]["o"]  # [48, 320, 32]
        out[b, :, j * HOUT : (j + 1) * HOUT, :] = o.transpose(2, 0, 1)
    return out


def kernel(q: np.ndarray, warped_feat: np.ndarray) -> np.ndarray:
    nc = get_nc()
    in_maps = make_in_maps(q, warped_feat)
    res = run_bass_kernel_spmd(nc, in_maps, list(range(NCORES)))
    return assemble(res.results)



# revision 2
# speedup vs baseline: 3.9108x; 3.9108x over previous
"""DAV_Block cost-volume kernel for Trainium2 (8 NeuronCores, SPMD).

Computes sim[b,d,h,w] = cosine similarity between 3x3xC patches of q and
warped_feat[..., d]:
    qq  = box3(sum_c q^2);  kk = box3(sum_c wf_d^2);  num = box3(sum_c q*wf_d)
    sim = num / (max(sqrt(qq),eps) * max(sqrt(kk),eps))

Sharding: 8 cores = b(2) x h-quarter(4).  Each core gets a 48-row h-slice
(+1 halo row each side, zeros at global edges) with all C, W, D.

Per-core dataflow (fp32 in, bf16 through the PE):
  partitions = (h-pair, c) = 128
  ACT: sq = wf^2          -> bf16
  DVE: pr = wf * q_bcast  -> bf16
  PE : banded ones lhsT [128, 48] (bf16 -- fp32r weights would lower to
       IndirectLoad weight DMAs whose 16-bit semaphore overflows at ~1k
       matmuls) performs channel-sum AND the 3-tap h-box in one
       accumulation chain; PSUM accumulates fp32.

Schedule (driven by a TimelineSim study of the previous version):
  * w is processed in 6 groups of [32,64,64,64,64,32] cols; per (group,t)
    one wf DMA [128p, cols x 32d] on the SP HWDGE ring (engines are
    in-order, so ACT/DVE never issue or wait on wf DMAs).
  * group 0 is 32 cols so its PSUM accs (4 banks) coexist with qq_acc
    (1 bank): phase A (qq) interleaves into group 0's t-loop with no
    PE stall.  Phase A's elementwise work runs on the otherwise-idle
    gpsimd engine so ACT/DVE start main-loop work immediately.
  * phase C (w-box + normalize + output DMA) is chunked; chunk k is
    emitted right after the group that supplies its last halo col and
    drained one op per t-iteration into the next group's loop, so no
    engine sees a burst.  Box/prod ops go to gpsimd (idle), recip/mul
    to DVE, sqrt to ACT.  Only a 33-col chunk remains after the last
    group -> ~10 us tail instead of ~50 us.

_build_nc(reps=N) emits the whole pipeline N times back-to-back into one
NEFF (constants + stage buffers hoisted; every rep recomputes the same
output).  reps>1 exists purely so a benchmark can time N device runs per
PJRT dispatch; kernel() always uses reps=1.
"""
import numpy as np
from contextlib import ExitStack

import concourse.bass as bass
from concourse import bacc
import concourse.tile as tile
from concourse import mybir
from concourse.bass_utils import run_bass_kernel_spmd

# Problem shape (hardcoded per contest contract)
B, C, H, W, D = 2, 64, 192, 320, 32
NCORES = 8
HQ = 4                 # h-quarters per batch
HOUT = H // HQ         # 48 out rows per core
HIN = HOUT + 2         # 50 input rows (1 halo each side)
NT = HIN // 2          # 25 h-pairs
J0 = HOUT              # center col of the banded weight pattern
GW = 2 * HOUT          # G width: cols [0, 96)
WHALF = W // 2         # 160
FSTG = (WHALF + 2) * D  # stage free size incl. 1 halo col each side: 162*32
MAXW = 64              # max group width (cols)
MAXCHUNK = 95          # max phase-C chunk width (cols)

# (start, width): first/last groups are 32 cols so their 4-bank PSUM accs
# leave room for qq_acc (group 0) / reuse freed banks (group 5)
GROUPS = [(0, 32), (32, 64), (96, 64), (160, 64), (224, 64), (288, 32)]
# phase-C chunks: (out-col start, out-col end, ready-after-group).  Chunk
# [a,b) needs stage cols a-1..b, i.e. the evacuation of the group holding
# col b (col 320 is the memset right edge halo).
CHUNKS = [(0, 95, 1), (95, 159, 2), (159, 223, 3), (223, 287, 4), (287, 320, 5)]

_NC_CACHE = {}


def _build_nc(reps=1):
    nc = bacc.Bacc(None, target_bir_lowering=False)
    wf_d = nc.declare_dram_parameter("wf", [C, HIN, W, D], mybir.dt.float32, isOutput=False)
    q_d = nc.declare_dram_parameter("q", [C, HIN, W], mybir.dt.float32, isOutput=False)
    g_d = nc.declare_dram_parameter("g", [128, GW], mybir.dt.float32, isOutput=False)
    o_d = nc.declare_dram_parameter("o", [HOUT, W, D], mybir.dt.float32, isOutput=True)

    f32 = mybir.dt.float32
    bf16 = mybir.dt.bfloat16
    SQ = mybir.ActivationFunctionType.Square

    with ExitStack() as ctx:
        tc = ctx.enter_context(tile.TileContext(nc))
        cpool = ctx.enter_context(tc.tile_pool(name="const", bufs=1))
        spool = ctx.enter_context(tc.tile_pool(name="stage", bufs=1))
        pool = ctx.enter_context(tc.tile_pool(name="work", bufs=6))
        epool = ctx.enter_context(tc.tile_pool(name="elem", bufs=4))

        # ---- constants (scalar HWDGE ring; sync ring is reserved for wf) ----
        g = cpool.tile([128, GW], f32)
        nc.scalar.dma_start(g[:], g_d[:])
        gb = cpool.tile([128, GW], bf16)
        nc.scalar.copy(gb[:], g[:])

        # q resident: [128p=(hpair,c), t, w] -- loaded per-t so phase A can
        # start as soon as the first rows land
        q_all = cpool.tile([128, NT, W], f32)
        q_src = q_d[:].rearrange("c (t hp) w -> hp c t w", hp=2)
        for t in range(NT):
            nc.scalar.dma_start(q_all[:, t, :], q_src[:, :, t, :])

        # ---- stage + scratch tensors ----
        num_stg = spool.tile([112, FSTG], f32)
        kk_stg = spool.tile([112, FSTG], f32)
        qq_stg = spool.tile([112, WHALF + 2], f32)
        qq_box = spool.tile([112, WHALF], f32)
        # phase-C chunk scratch (persistent; chunks use [quadrant, :wd*D])
        bnum_s = spool.tile([112, MAXCHUNK * D], f32)
        bkk_s = spool.tile([112, MAXCHUNK * D], f32)
        p1_s = spool.tile([112, MAXCHUNK * D], f32)
        p2_s = spool.tile([112, MAXCHUNK * D], f32)

        # edge halos (independent of everything; gpsimd is idle at t=0).
        # One-time: reps only rewrite the center cols.
        nc.gpsimd.memset(num_stg[0:HOUT, 0:D], 0.0)
        nc.gpsimd.memset(kk_stg[0:HOUT, 0:D], 0.0)
        nc.gpsimd.memset(num_stg[64 : 64 + HOUT, (WHALF + 1) * D : FSTG], 0.0)
        nc.gpsimd.memset(kk_stg[64 : 64 + HOUT, (WHALF + 1) * D : FSTG], 0.0)
        nc.gpsimd.memset(qq_stg[0:HOUT, 0:1], 0.0)
        nc.gpsimd.memset(qq_stg[64 : 64 + HOUT, WHALF + 1 : WHALF + 2], 0.0)
        # recip/sqrt below run on partitions [0:112] (base 0) because the
        # custom DVE ISA op reciprocal_approx_fast silently no-ops when its
        # AP partition base is 64 (HW-probed; CoreSim models it fine).  Fill
        # the dead 48:64 band so those lanes read initialized data.
        nc.gpsimd.memset(p1_s[0:112, :], 1.0)
        nc.gpsimd.memset(p2_s[0:112, :], 1.0)

        for _rep in range(reps):
            _emit_body(nc, tc, pool, epool, gb, q_all,
                       num_stg, kk_stg, qq_stg, qq_box,
                       bnum_s, bkk_s, p1_s, p2_s, wf_d, o_d)

    nc.compile()
    return nc


def _emit_body(nc, tc, pool, epool, gb, q_all, num_stg, kk_stg, qq_stg,
               qq_box, bnum_s, bkk_s, p1_s, p2_s, wf_d, o_d):
    """One full pipeline: phases A (qq), B (groups), C (normalize+store)."""
    f32 = mybir.dt.float32
    bf16 = mybir.dt.bfloat16
    SQ = mybir.ActivationFunctionType.Square

    # queued (weight, op) phase-C closures.  Engines are in-order, so a
    # queued op that still waits on its predecessor blocks everything
    # behind it on that engine: ops must be spaced out so each link of
    # the serial chain finishes before the next engine reaches it.
    # 1 credit accrues per t-iteration; heavy (>=33 col) ops cost 2.
    pending = []
    drain_credit = [0]

    def drain(credits):
        drain_credit[0] += credits
        while pending and drain_credit[0] >= pending[0][0]:
            w, op = pending.pop(0)
            drain_credit[0] -= w
            op()

    def emit_chunk(a, b, tail=False, soff=0):
        """Box-w + normalize + output DMA for out cols [a, b).

        Queues (weight, closure) pairs on `pending` (returns the op
        list if tail).  [a,b) must lie in one w-half.  Box adds and
        prod run on gpsimd, where in-order blocking is harmless (no
        main-loop work queues behind them).  The normalize (sqrt ->
        recip -> mul) is emitted in 16-col strips so the ACT/DVE
        links each wait at most ~0.5us -- a big queued op that waits
        would stall every main-loop op behind it on that engine.
        """
        qd = a // WHALF
        wo = a - WHALF * qd
        wd = b - a
        ps = slice(64 * qd, 64 * qd + HOUT)
        n = wd * D
        ss = slice(soff, soff + n)
        ops = []
        box_eng = nc.vector
        bnum_eng = nc.gpsimd

        bw = 2 if wd >= 33 else (0 if wd <= 4 else 1)

        def box2(dst, src, eng):
            out = []
            out.append((bw, lambda: eng.tensor_add(
                dst[ps, ss],
                src[ps, wo * D : wo * D + n],
                src[ps, (wo + 2) * D : (wo + 2) * D + n])))
            out.append((bw, lambda: eng.tensor_add(
                dst[ps, ss], dst[ps, ss],
                src[ps, (wo + 1) * D : (wo + 1) * D + n])))
            return out

        bk = box2(bkk_s, kk_stg, box_eng)
        bn = box2(bnum_s, num_stg, bnum_eng)
        ops += [bk[0], bn[0], bk[1], bn[1]]
        sws = 0 if wd <= 4 else 1
        for s0 in range(0, wd, 32):
            sw = min(32, wd - s0)
            m = sw * D
            st = slice(soff + s0 * D, soff + s0 * D + m)
            ops.append((sws, lambda st=st, s0=s0, sw=sw, m=m:
                nc.gpsimd.tensor_mul(
                    p1_s[ps, st].rearrange("p (w d) -> p w d", d=D),
                    bkk_s[ps, st].rearrange("p (w d) -> p w d", d=D),
                    qq_box[ps, wo + s0 : wo + s0 + sw]
                    .unsqueeze(-1).broadcast_to([HOUT, sw, D]))))
            ops.append((sws, lambda st=st: nc.scalar.activation(
                p2_s[0:112, st], p1_s[0:112, st],
                mybir.ActivationFunctionType.Sqrt)))
            ops.append((sws, lambda st=st: nc.vector.reciprocal_approx_fast(
                p1_s[0:112, st], p2_s[0:112, st])))
            ops.append((sws, lambda st=st: nc.vector.tensor_mul(
                p2_s[ps, st], bnum_s[ps, st], p1_s[ps, st])))
        ops.append((1, lambda: nc.gpsimd.dma_start(
            o_d[:, a:b, :],
            p2_s[ps, ss].rearrange("p (w d) -> p w d", d=D))))
        if tail:
            return [op for _, op in ops]
        pending.extend(ops)
        return None

    def run_group(gi, qq_work, mpsum, qq_acc=None):
        wstart, wd = GROUPS[gi]
        nfree = wd * D
        nch = nfree // 512
        acc_num = mpsum.tile([128, nfree], f32, tag=f"acc_num{wd}")
        acc_kk = mpsum.tile([128, nfree], f32, tag=f"acc_kk{wd}")
        for t in range(NT):
            wf_t = pool.tile([128, MAXW * D], f32, tag="wf")
            src = (
                wf_d[:]
                .rearrange("c (t hp) w d -> t hp c w d", hp=2)[t]
                [:, :, wstart : wstart + wd, :]
            )
            nc.sync.dma_start(
                wf_t[:, 0:nfree].rearrange("p (w d) -> p w d", d=D), src
            )

            sq_t = epool.tile([128, MAXW * D], bf16, tag="sq")
            nc.scalar.activation(sq_t[:, 0:nfree], wf_t[:, 0:nfree], SQ)

            pr_t = epool.tile([128, MAXW * D], bf16, tag="pr")
            q_b = (
                q_all[:, t, wstart : wstart + wd]
                .unsqueeze(-1)
                .broadcast_to([128, wd, D])
            )
            nc.vector.tensor_mul(
                pr_t[:, 0:nfree].rearrange("p (w d) -> p w d", d=D),
                wf_t[:, 0:nfree].rearrange("p (w d) -> p w d", d=D),
                q_b,
            )

            lhsT = gb[:, J0 - 2 * t : J0 - 2 * t + HOUT]
            first, last = (t == 0), (t == NT - 1)
            for ch in range(nch):
                sl = slice(512 * ch, 512 * (ch + 1))
                nc.tensor.matmul(acc_num[0:HOUT, sl], lhsT, pr_t[:, sl],
                                 start=first, stop=last)
                nc.tensor.matmul(acc_kk[0:HOUT, sl], lhsT, sq_t[:, sl],
                                 start=first, stop=last)
            if qq_work:
                # phase A rides along group 0: q DMAs run well ahead of
                # wf DMAs, so these never block the in-order PE
                sqq = epool.tile([128, W], bf16, tag="sqq")
                nc.gpsimd.tensor_mul(sqq[:], q_all[:, t, :], q_all[:, t, :])
                nc.tensor.matmul(qq_acc[0:HOUT, :], lhsT, sqq[:],
                                 start=first, stop=last)
            drain(1)

        # evacuate: psum [48, nfree] -> stage quadrant (both on ACT --
        # DVE is the scarcer engine)
        qd = wstart // WHALF
        wo = wstart - WHALF * qd
        pbase = 64 * qd
        foff = (1 + wo) * D
        nc.scalar.copy(num_stg[pbase : pbase + HOUT, foff : foff + nfree],
                       acc_num[0:HOUT, :])
        nc.scalar.copy(kk_stg[pbase : pbase + HOUT, foff : foff + nfree],
                       acc_kk[0:HOUT, :])

    # ---- main loop ----
    with tc.tile_pool(name="qq_psum", bufs=1, space="PSUM") as qpsum, \
         tc.tile_pool(name="mm32_psum", bufs=1, space="PSUM") as mpsum:
        qq_acc = qpsum.tile([128, W], f32)
        run_group(0, True, mpsum, qq_acc)
        # qq: evacuate (ACT reaches this ~40 us in; the qq matmul chain
        # finished at ~12 us, so no stall), halos + box on gpsimd
        nc.scalar.copy(qq_stg[0:HOUT, 1 : WHALF + 1], qq_acc[0:HOUT, 0:WHALF])
        nc.scalar.copy(qq_stg[64 : 64 + HOUT, 1 : WHALF + 1],
                       qq_acc[0:HOUT, WHALF:W])
        nc.gpsimd.tensor_copy(qq_stg[0:HOUT, WHALF + 1 : WHALF + 2],
                              qq_stg[64 : 64 + HOUT, 1:2])
        nc.gpsimd.tensor_copy(qq_stg[64 : 64 + HOUT, 0:1],
                              qq_stg[0:HOUT, WHALF : WHALF + 1])
        for qs in (slice(0, HOUT), slice(64, 64 + HOUT)):
            nc.gpsimd.tensor_add(qq_box[qs, :], qq_stg[qs, 0:WHALF],
                                 qq_stg[qs, 2 : WHALF + 2])
            nc.gpsimd.tensor_add(qq_box[qs, :], qq_box[qs, :],
                                 qq_stg[qs, 1 : WHALF + 1])

    with tc.tile_pool(name="mm64_psum", bufs=1, space="PSUM") as mpsum:
        for gi in (1, 2, 3, 4):
            run_group(gi, False, mpsum)
            if gi == 3:
                # interface halos between the two w-halves (cols 159/160
                # both evacuated now; needed by the chunks ending at 223)
                nc.scalar.copy(num_stg[0:HOUT, (WHALF + 1) * D : FSTG],
                               num_stg[64 : 64 + HOUT, D : 2 * D])
                nc.scalar.copy(kk_stg[0:HOUT, (WHALF + 1) * D : FSTG],
                               kk_stg[64 : 64 + HOUT, D : 2 * D])
                nc.scalar.copy(num_stg[64 : 64 + HOUT, 0:D],
                               num_stg[0:HOUT, WHALF * D : (WHALF + 1) * D])
                nc.scalar.copy(kk_stg[64 : 64 + HOUT, 0:D],
                               kk_stg[0:HOUT, WHALF * D : (WHALF + 1) * D])
            for a, bnd, rdy in CHUNKS:
                if rdy == gi:
                    if a < WHALF < bnd:
                        emit_chunk(a, WHALF)
                        emit_chunk(WHALF, bnd)
                    else:
                        emit_chunk(a, bnd)

    with tc.tile_pool(name="mm32b_psum", bufs=1, space="PSUM") as mpsum:
        run_group(5, False, mpsum)

    while pending:
        pending.pop(0)[1]()
    # tail: two parallel chains on complementary engine flavors
    ops_a = emit_chunk(287, 303, tail=True)
    ops_b = emit_chunk(303, 320, tail=True, soff=16 * D)
    for op_a, op_b in zip(ops_a, ops_b):
        op_a()
        op_b()


def _g_pattern() -> np.ndarray:
    """g[p=(hp*64+c), j] = 1 iff j - J0 in {hp-2, hp-1, hp}."""
    g = np.zeros((128, GW), dtype=np.float32)
    for hp in range(2):
        for dj in (hp - 2, hp - 1, hp):
            j = J0 + dj
            if 0 <= j < GW:
                g[hp * 64 : (hp + 1) * 64, j] = 1.0
    return g


def get_nc(reps=1):
    if reps not in _NC_CACHE:
        _NC_CACHE[reps] = _build_nc(reps)
    return _NC_CACHE[reps]


def make_in_maps(q: np.ndarray, warped_feat: np.ndarray):
    """Marshal full inputs into 8 per-core input maps."""
    q = np.asarray(q, dtype=np.float32)
    wf = np.asarray(warped_feat, dtype=np.float32)
    g = _g_pattern()
    in_maps = []
    for core in range(NCORES):
        b, j = divmod(core, HQ)
        h0 = j * HOUT - 1          # inclusive, may be -1
        h1 = j * HOUT + HOUT + 1   # exclusive, may be H+1
        lo_pad = 1 if h0 < 0 else 0
        hi_pad = 1 if h1 > H else 0
        hs = slice(h0 + lo_pad, h1 - hi_pad)
        q_c = np.zeros((C, HIN, W), dtype=np.float32)
        q_c[:, lo_pad : HIN - hi_pad, :] = q[b][:, hs, :]
        wf_c = np.zeros((C, HIN, W, D), dtype=np.float32)
        wf_c[:, lo_pad : HIN - hi_pad, :, :] = wf[b][:, hs, :, :]
        in_maps.append({"wf": wf_c, "q": q_c, "g": g})
    return in_maps


def assemble(results) -> np.ndarray:
    out = np.empty((B, D, H, W), dtype=np.float32)
    for core in range(NCORES):
        b, j = divmod(core, HQ)
        o = results[core]["o"]  # [48, 320, 32]
        out[b, :, j * HOUT : (j + 1) * HOUT, :] = o.transpose(2, 0, 1)
    return out


def kernel(q: np.ndarray, warped_feat: np.ndarray) -> np.ndarray:
    nc = get_nc()
    in_maps = make_in_maps(q, warped_feat)
    res = run_bass_kernel_spmd(nc, in_maps, list(range(NCORES)))
    return assemble(res.results)
